# revision 6
# baseline (speedup 1.0000x reference)
"""BASE-layer MoE kernel for Trainium2, expert-parallel across 8 NeuronCores.

Strategy (matches the expert-parallel sharding hint):
  - Routing/balanced assignment is replicated (computed once with the exact
    same jax ops as the reference so the permutation matches bit-for-bit),
    tokens are permuted into [E, C, D] on the host, and each of the 8 cores
    runs its own expert's 2-layer residual FFN (LN -> ff1 -> relu -> ff2 ->
    residual, then sigmoid-gated by the token/centroid affinity).
  - ln_gamma/ln_beta are folded into W1/b1 on the host (exact algebra):
      W1_eff = W1 * gamma[None, :],  b1_eff = b1 + W1 @ beta
  - Matmuls run in bf16 (fp32 accumulation in PSUM); LN statistics, the
    residual stream and the alpha gate stay fp32.
"""

import numpy as np

import concourse.bass as bass
import concourse.mybir as mybir
import concourse.tile as tile
from concourse.bass_utils import run_bass_kernel_spmd

S, B, D, F, E, L = 2048, 4, 1024, 4096, 8, 2
EPS = 1e-5
T = S * B
C = T // E
P = 128
DT = D // P   # 8 d tiles
FT = F // P   # 32 f tiles
CT = C // P   # 8 c tiles
F32 = mybir.dt.float32
BF16 = mybir.dt.bfloat16

# ---------------------------------------------------------------------------
# Workaround: this walrus build rejects >1 sync wait on one instruction
# ("Too many sync wait commands"), but Tile routinely attaches several. After
# tracing, split excess waits onto same-engine NOPs inserted just before the
# instruction — the engine stalls at the NOPs instead, semantics unchanged.
# ---------------------------------------------------------------------------
_MAX_WAITS = 1


def _split_multi_waits(nc, limit=_MAX_WAITS):
    n_split = 0
    for f in nc.m.functions:
        for bb in f.blocks:
            insts = bb.instructions
            out = []
            changed = False
            for ins in insts:
                si = getattr(ins, "sync_info", None)
                if si is not None and si.on_wait and len(si.on_wait) > limit:
                    waits = list(si.on_wait)
                    head, tail = waits[:-limit], waits[-limit:]
                    for i in range(0, len(head), limit):
                        n_split += 1
                        nop = mybir.InstNoOp(
                            name=f"waitsplit-{n_split}",
                            engine=ins.engine,
                            text_hint="waitsplit",
                            bass_nofuse=True,
                        )
                        nop.sync_info = mybir.SyncInfo(
                            on_wait=head[i : i + limit], on_update=[]
                        )
                        out.append(nop)
                    ins.sync_info = mybir.SyncInfo(
                        on_wait=tail, on_update=list(si.on_update or [])
                    )
                    changed = True
                out.append(ins)
            if changed:
                bb.instructions = out
    return n_split


# ---------------------------------------------------------------------------
# Device program (identical on all 8 cores; per-core data differs)
# ---------------------------------------------------------------------------
def _bcast_ap(ap, parts=P):
    """Partition-stride-0 broadcast of a 1-D DRAM AP to [parts, n]."""
    return bass.AP(tensor=ap.tensor, offset=ap.offset, ap=[[0, parts], *ap.ap])


def build_bass():
    nc = bass.Bass()
    x_d = nc.declare_dram_parameter("x", [C, D], F32, isOutput=False)
    w1_d = nc.declare_dram_parameter("w1", [L, FT, P, DT, P], BF16, isOutput=False)
    b1_d = nc.declare_dram_parameter("b1", [L, P, FT], F32, isOutput=False)
    w2_d = nc.declare_dram_parameter("w2", [L, P, FT, D], BF16, isOutput=False)
    b2_d = nc.declare_dram_parameter("b2", [L, D], F32, isOutput=False)
    cen_d = nc.declare_dram_parameter("cen", [D], F32, isOutput=False)
    y_d = nc.declare_dram_parameter("y", [C, D], F32, isOutput=True)

    with tile.TileContext(nc) as tc:
        import contextlib

        with contextlib.ExitStack() as ctx:
            singles = ctx.enter_context(tc.tile_pool(name="singles", bufs=1))
            xpool = ctx.enter_context(tc.tile_pool(name="xpool", bufs=1))
            htpool = ctx.enter_context(tc.tile_pool(name="htpool", bufs=1))
            h1pool = ctx.enter_context(tc.tile_pool(name="h1pool", bufs=1))
            w2pool = ctx.enter_context(tc.tile_pool(name="w2pool", bufs=1))
            w1pool = ctx.enter_context(tc.tile_pool(name="w1pool", bufs=4))
            tmps = ctx.enter_context(tc.tile_pool(name="tmps", bufs=3))
            stats = ctx.enter_context(tc.tile_pool(name="stats", bufs=4))
            ps1 = ctx.enter_context(tc.tile_pool(name="ps1", bufs=3, space="PSUM"))
            ps2 = ctx.enter_context(tc.tile_pool(name="ps2", bufs=3, space="PSUM"))

            # --- constants / persistent state ---
            eps_t = singles.tile([P, 1], F32)
            nc.vector.memset(eps_t, EPS)
            cen_b = singles.tile([P, D], F32)
            nc.gpsimd.dma_start(out=cen_b, in_=_bcast_ap(cen_d[:]))
            alpha = singles.tile([P, CT], F32)
            b1_sb = singles.tile([P, L, FT], F32)
            for l in range(L):
                nc.sync.dma_start(out=b1_sb[:, l, :], in_=b1_d[l])
            b2_b = singles.tile([P, L, D], F32)
            for l in range(L):
                nc.gpsimd.dma_start(out=b2_b[:, l, :], in_=_bcast_ap(b2_d[l]))

            x_sb = xpool.tile([P, CT, D], F32)
            for ct in range(CT):
                nc.sync.dma_start(
                    out=x_sb[:, ct, :], in_=x_d[ct * P : (ct + 1) * P, :]
                )

            # --- alpha = sigmoid(x0 . centroid) ---
            for ct in range(CT):
                tmp = tmps.tile([P, D], F32, tag="alpha_tmp")
                nc.vector.tensor_mul(out=tmp, in0=x_sb[:, ct, :], in1=cen_b)
                dot = stats.tile([P, 1], F32, tag="alpha_dot")
                nc.vector.reduce_sum(out=dot, in_=tmp, axis=mybir.AxisListType.X)
                nc.scalar.activation(
                    out=alpha[:, ct : ct + 1],
                    in_=dot,
                    func=mybir.ActivationFunctionType.Sigmoid,
                )

            ht = htpool.tile([P, DT, C], BF16)   # h^T: [d_p, dt, c]
            h1 = h1pool.tile([P, FT, C // 2], BF16)  # per c-half: [f_p, ft, c]
            w2_sb = w2pool.tile([P, FT, D], BF16)

            for l in range(L):
                # --- LayerNorm (token-major) + transpose to ht ---
                for ct in range(CT):
                    st = stats.tile([P, 2, 6], F32, tag="bn_st")
                    xin = x_sb[:, ct, :].rearrange("p (s q) -> p s q", s=2)
                    for s in range(2):
                        nc.vector.bn_stats(out=st[:, s, :], in_=xin[:, s, :])
                    mv = stats.tile([P, 2], F32, tag="bn_mv")
                    nc.vector.bn_aggr(out=mv, in_=st)
                    nc.scalar.activation(
                        out=mv[:, 1:2],
                        in_=mv[:, 1:2],
                        func=mybir.ActivationFunctionType.Sqrt,
                        bias=eps_t,
                        scale=1.0,
                    )
                    nc.vector.reciprocal(out=mv[:, 1:2], in_=mv[:, 1:2])
                    h_tm = tmps.tile([P, D], BF16, tag="h_tm")
                    nc.vector.tensor_scalar(
                        out=h_tm,
                        in0=x_sb[:, ct, :],
                        scalar1=mv[:, 0:1],
                        scalar2=mv[:, 1:2],
                        op0=mybir.AluOpType.subtract,
                        op1=mybir.AluOpType.mult,
                    )
                    for dt in range(DT):
                        nc.sync.dma_start(
                            out=ht[:, dt, ct * P : (ct + 1) * P],
                            in_=h_tm[:, dt * P : (dt + 1) * P],
                            transpose=True,
                        )

                # --- W2 for this layer ---
                nc.sync.dma_start(out=w2_sb, in_=w2_d[l])

                for ch in range(2):  # c halves of 512
                    csl = slice(ch * (C // 2), (ch + 1) * (C // 2))
                    # --- ff1: h1[f, c] = relu(W1eff^T.T @ ht + b1) ---
                    for ft in range(FT):
                        w1c = w1pool.tile([P, DT, P], BF16, tag="w1c")
                        nc.sync.dma_start(out=w1c, in_=w1_d[l, ft])
                        pt = ps1.tile([P, 512], F32, tag="ps1")
                        for dt in range(DT):
                            nc.tensor.matmul(
                                pt,
                                lhsT=w1c[:, dt, :],
                                rhs=ht[:, dt, csl],
                                start=(dt == 0),
                                stop=(dt == DT - 1),
                            )
                        nc.scalar.activation(
                            out=h1[:, ft, :],
                            in_=pt,
                            func=mybir.ActivationFunctionType.Relu,
                            bias=b1_sb[:, l, ft : ft + 1],
                            scale=1.0,
                        )
                    # --- ff2 + residual (+ b2) ---
                    for ctl in range(CT // 2):
                        ct = ch * (CT // 2) + ctl
                        for dh in range(2):
                            dsl = slice(dh * 512, (dh + 1) * 512)
                            pt2 = ps2.tile([P, 512], F32, tag="ps2")
                            for ft in range(FT):
                                nc.tensor.matmul(
                                    pt2,
                                    lhsT=h1[:, ft, ctl * P : (ctl + 1) * P],
                                    rhs=w2_sb[:, ft, dsl],
                                    start=(ft == 0),
                                    stop=(ft == FT - 1),
                                )
                            nc.vector.tensor_add(
                                out=x_sb[:, ct, dsl],
                                in0=x_sb[:, ct, dsl],
                                in1=pt2,
                            )
                            nc.vector.tensor_add(
                                out=x_sb[:, ct, dsl],
                                in0=x_sb[:, ct, dsl],
                                in1=b2_b[:, l, dsl],
                            )

            # --- y = alpha * x ---
            for ct in range(CT):
                nc.vector.tensor_scalar_mul(
                    out=x_sb[:, ct, :],
                    in0=x_sb[:, ct, :],
                    scalar1=alpha[:, ct : ct + 1],
                )
                nc.sync.dma_start(
                    out=y_d[ct * P : (ct + 1) * P, :], in_=x_sb[:, ct, :]
                )
    _split_multi_waits(nc)
    return nc


_NC_CACHE = None


def _get_nc():
    global _NC_CACHE
    if _NC_CACHE is None:
        _NC_CACHE = build_bass()
    return _NC_CACHE


# ---------------------------------------------------------------------------
# Host side: routing (replicated, bit-exact with the reference) + sharding
# ---------------------------------------------------------------------------
def _routing_perm(features, centroids):
    # Replicates the reference's _balanced_assignment with the exact same jax
    # ops, pinned to the CPU backend: the reference itself can only run on
    # CPU jax (stable sort doesn't compile for the neuron backend), so CPU
    # numerics are the ones the permutation must match bit-for-bit.
    import jax
    import jax.numpy as jnp

    with jax.default_device(jax.devices("cpu")[0]):
        feats = jnp.asarray(features)
        cents = jnp.asarray(centroids)
        aff = jax.lax.stop_gradient(feats) @ jax.lax.stop_gradient(cents).T
        aff = jnp.nan_to_num(aff)
        capacity = feats.shape[0] // cents.shape[0]
        order = jnp.argsort(-aff.max(axis=1))
        aff_ord = aff[order]

        def step(counts, row):
            masked = jnp.where(counts < capacity, row, -jnp.inf)
            e = jnp.argmax(masked).astype(jnp.int32)
            return counts.at[e].add(1), e

        _, assign_ord = jax.lax.scan(
            step, jnp.zeros(cents.shape[0], jnp.int32), aff_ord
        )
        assign = jnp.zeros(feats.shape[0], jnp.int32).at[order].set(assign_ord)
        return np.asarray(jnp.argsort(assign))


def _prep_core_inputs(xr, centroids, ln_gamma, ln_beta, W1, b1, W2, b2):
    """Per-core input maps; folds gamma/beta into W1/b1 and pre-tiles weights."""
    maps = []
    for e in range(E):
        m = {"x": np.ascontiguousarray(xr[e])}
        w1s = np.empty((L, FT, P, DT, P), np.float32)
        w2s = np.empty((L, P, FT, D), np.float32)
        b1s = np.empty((L, P, FT), np.float32)
        for l in range(L):
            g = ln_gamma[l, e]
            bt = ln_beta[l, e]
            w1_eff = W1[l, e] * g[None, :]          # [F, D]
            b1_eff = b1[l, e] + W1[l, e] @ bt       # [F]
            # lhsT tiles: w1s[l, ft, p_d, dt, j_f] = w1_eff[ft*P+j, dt*P+p]
            w1s[l] = w1_eff.reshape(FT, P, DT, P).transpose(0, 3, 2, 1)
            # w2s[l, p_f, ft, d] = W2[l,e][d, ft*P+p]
            w2s[l] = W2[l, e].T.reshape(FT, P, D).transpose(1, 0, 2)
            b1s[l] = b1_eff.reshape(FT, P).T
        import ml_dtypes

        m["w1"] = w1s.astype(ml_dtypes.bfloat16)
        m["w2"] = w2s.astype(ml_dtypes.bfloat16)
        m["b1"] = b1s
        m["b2"] = np.ascontiguousarray(b2[:, e, :]).astype(np.float32)
        m["cen"] = np.ascontiguousarray(centroids[e]).astype(np.float32)
        maps.append(m)
    return maps


def kernel(
    input_features,
    centroids,
    ln_gamma,
    ln_beta,
    W1,
    b1,
    W2,
    b2,
    input_ids=None,
    _trace=False,
    _tmpdir=None,
):
    input_features = np.asarray(input_features, np.float32)
    centroids = np.asarray(centroids, np.float32)
    ln_gamma = np.asarray(ln_gamma, np.float32)
    ln_beta = np.asarray(ln_beta, np.float32)
    W1 = np.asarray(W1, np.float32)
    b1 = np.asarray(b1, np.float32)
    W2 = np.asarray(W2, np.float32)
    b2 = np.asarray(b2, np.float32)

    feats = input_features.reshape(T, D)
    perm = _routing_perm(feats, centroids)
    xr = feats[perm].reshape(E, C, D)

    maps = _prep_core_inputs(xr, centroids, ln_gamma, ln_beta, W1, b1, W2, b2)
    nc = _get_nc()
    res = run_bass_kernel_spmd(
        nc, maps, list(range(E)), trace=_trace, tmpdir=_tmpdir
    )
    y = np.concatenate([res.results[e]["y"] for e in range(E)], axis=0)  # [T, D]
    out = np.zeros((T, D), np.float32)
    out[perm] = y
    out = out.reshape(input_features.shape)
    if _trace:
        return out, res
    return out


# revision 11
# speedup vs baseline: 1.1156x; 1.1156x over previous
"""BASE-layer MoE kernel for Trainium2, expert-parallel across 8 NeuronCores.

Strategy (matches the expert-parallel sharding hint):
  - Routing/balanced assignment is replicated (computed once with the exact
    same jax ops as the reference so the permutation matches bit-for-bit),
    tokens are permuted into [E, C, D] on the host, and each of the 8 cores
    runs its own expert's 2-layer residual FFN (LN -> ff1 -> relu -> ff2 ->
    residual, then sigmoid-gated by the token/centroid affinity).
  - ln_gamma/ln_beta are folded into W1/b1 on the host (exact algebra):
      W1_eff = W1 * gamma[None, :],  b1_eff = b1 + W1 @ beta
  - Matmuls run in bf16 (fp32 accumulation in PSUM); LN statistics, the
    residual stream and the alpha gate stay fp32.
"""

import numpy as np

import concourse.bass as bass
import concourse.mybir as mybir
import concourse.tile as tile
from concourse.bass_utils import run_bass_kernel_spmd

S, B, D, F, E, L = 2048, 4, 1024, 4096, 8, 2
EPS = 1e-5
T = S * B
C = T // E
P = 128
DT = D // P   # 8 d tiles
FT = F // P   # 32 f tiles
CT = C // P   # 8 c tiles
F32 = mybir.dt.float32
BF16 = mybir.dt.bfloat16

# ---------------------------------------------------------------------------
# Workaround: this walrus build rejects >1 sync wait on one instruction
# ("Too many sync wait commands"), but Tile routinely attaches several. After
# tracing, split excess waits onto same-engine NOPs inserted just before the
# instruction — the engine stalls at the NOPs instead, semantics unchanged.
# ---------------------------------------------------------------------------
_MAX_WAITS = 1


def _split_multi_waits(nc, limit=_MAX_WAITS):
    n_split = 0
    for f in nc.m.functions:
        for bb in f.blocks:
            insts = bb.instructions
            out = []
            changed = False
            for ins in insts:
                si = getattr(ins, "sync_info", None)
                if si is not None and si.on_wait and len(si.on_wait) > limit:
                    waits = list(si.on_wait)
                    head, tail = waits[:-limit], waits[-limit:]
                    for i in range(0, len(head), limit):
                        n_split += 1
                        nop = mybir.InstNoOp(
                            name=f"waitsplit-{n_split}",
                            engine=ins.engine,
                            text_hint="waitsplit",
                            bass_nofuse=True,
                        )
                        nop.sync_info = mybir.SyncInfo(
                            on_wait=head[i : i + limit], on_update=[]
                        )
                        out.append(nop)
                    ins.sync_info = mybir.SyncInfo(
                        on_wait=tail, on_update=list(si.on_update or [])
                    )
                    changed = True
                out.append(ins)
            if changed:
                bb.instructions = out
    return n_split


# ---------------------------------------------------------------------------
# Device program (identical on all 8 cores; per-core data differs)
# ---------------------------------------------------------------------------
def _bcast_ap(ap, parts=P):
    """Partition-stride-0 broadcast of a 1-D DRAM AP to [parts, n]."""
    return bass.AP(tensor=ap.tensor, offset=ap.offset, ap=[[0, parts], *ap.ap])


def build_bass(split_waits=True):
    nc = bass.Bass()
    x_d = nc.declare_dram_parameter("x", [C, D], F32, isOutput=False)
    w1_d = nc.declare_dram_parameter("w1", [L, FT, P, DT, P], BF16, isOutput=False)
    b1_d = nc.declare_dram_parameter("b1", [L, P, FT], F32, isOutput=False)
    w2_d = nc.declare_dram_parameter("w2", [L, P, FT, D], BF16, isOutput=False)
    b2_d = nc.declare_dram_parameter("b2", [L, D], F32, isOutput=False)
    cen_d = nc.declare_dram_parameter("cen", [D], F32, isOutput=False)
    y_d = nc.declare_dram_parameter("y", [C, D], F32, isOutput=True)

    with tile.TileContext(nc) as tc:
        import contextlib

        with contextlib.ExitStack() as ctx:
            singles = ctx.enter_context(tc.tile_pool(name="singles", bufs=1))
            xpool = ctx.enter_context(tc.tile_pool(name="xpool", bufs=1))
            htpool = ctx.enter_context(tc.tile_pool(name="htpool", bufs=1))
            h1pool = ctx.enter_context(tc.tile_pool(name="h1pool", bufs=1))
            w2pool = ctx.enter_context(tc.tile_pool(name="w2pool", bufs=1))
            w1pool = ctx.enter_context(tc.tile_pool(name="w1pool", bufs=16))
            tmps = ctx.enter_context(tc.tile_pool(name="tmps", bufs=3))
            stats = ctx.enter_context(tc.tile_pool(name="stats", bufs=6))
            ps1 = ctx.enter_context(tc.tile_pool(name="ps1", bufs=3, space="PSUM"))
            ps2 = ctx.enter_context(tc.tile_pool(name="ps2", bufs=3, space="PSUM"))

            # --- x loads first: they gate the LN -> transpose -> ff1 chain.
            # Split per ct and per d-half so per-queue latency stays low.
            xs = []
            for ct in range(CT):
                xt = xpool.tile([P, D], F32, tag=f"x{ct}")
                for dh in range(2):
                    nc.sync.dma_start(
                        out=xt[:, dh * 512 : (dh + 1) * 512],
                        in_=x_d[ct * P : (ct + 1) * P, dh * 512 : (dh + 1) * 512],
                    )
                xs.append(xt)

            # --- constants (small; issued on gpsimd to stay off the SP path)
            eps_t = singles.tile([P, 1], F32)
            nc.vector.memset(eps_t, EPS)
            cen_b = singles.tile([P, D], F32)
            nc.gpsimd.dma_start(out=cen_b, in_=_bcast_ap(cen_d[:]))
            alpha = singles.tile([P, CT], F32)
            b1_sb = singles.tile([P, L, FT], F32)
            for l in range(L):
                nc.sync.dma_start(out=b1_sb[:, l, :], in_=b1_d[l])
            b2_b = singles.tile([P, L, D], F32)
            for l in range(L):
                nc.gpsimd.dma_start(out=b2_b[:, l, :], in_=_bcast_ap(b2_d[l]))

            ht = htpool.tile([P, DT, C], BF16)       # h^T: [d_p, dt, c]
            h1 = h1pool.tile([P, FT, C // 2], BF16)  # per c-half: [f_p, ft, c]
            w2_sb = w2pool.tile([P, FT, D], BF16)

            # Pre-issue the first w1 chunks (= pool depth, so no slot waits)
            # and layer-0's w2 ahead of the LN/transpose section: SP executes
            # its DMA stream in order, and a dependent transpose ahead of
            # these would head-of-line-block them for tens of us.
            w1_pre = []
            for ft in range(16):
                w1c = w1pool.tile([P, DT, P], BF16, tag="w1c")
                nc.sync.dma_start(out=w1c, in_=w1_d[0, ft])
                w1_pre.append(w1c)
            nc.sync.dma_start(out=w2_sb, in_=w2_d[0])

            def emit_ln(l, ct):
                """LayerNorm of x[ct] (token-major) into h_tm, then transpose
                the 8 [128,128] blocks into ht. Stats on DVE, apply on ACT."""
                st = stats.tile([P, 2, 6], F32, tag="bn_st")
                xin = xs[ct].rearrange("p (s q) -> p s q", s=2)
                for s in range(2):
                    nc.vector.bn_stats(out=st[:, s, :], in_=xin[:, s, :])
                mv = stats.tile([P, 2], F32, tag="bn_mv")
                nc.vector.bn_aggr(out=mv, in_=st)
                nc.scalar.activation(
                    out=mv[:, 1:2],
                    in_=mv[:, 1:2],
                    func=mybir.ActivationFunctionType.Sqrt,
                    bias=eps_t,
                    scale=1.0,
                )
                nc.vector.reciprocal(out=mv[:, 1:2], in_=mv[:, 1:2])
                nb = stats.tile([P, 1], F32, tag="negmr")
                nc.vector.tensor_scalar(
                    out=nb,
                    in0=mv[:, 0:1],
                    scalar1=mv[:, 1:2],
                    scalar2=-1.0,
                    op0=mybir.AluOpType.mult,
                    op1=mybir.AluOpType.mult,
                )
                h_tm = tmps.tile([P, D], BF16, tag="h_tm")
                nc.scalar.activation(
                    out=h_tm,
                    in_=xs[ct],
                    func=mybir.ActivationFunctionType.Identity,
                    bias=nb,
                    scale=mv[:, 1:2],
                )
                for dt in range(DT):
                    nc.sync.dma_start(
                        out=ht[:, dt, ct * P : (ct + 1) * P],
                        in_=h_tm[:, dt * P : (dt + 1) * P],
                        transpose=True,
                    )

            for l in range(L):
                for ct in range(CT):
                    emit_ln(l, ct)
                    if l == 0:
                        # alpha = sigmoid(x0 . centroid); emitted per-ct right
                        # after its LN so it stays off the startup critical path
                        junk = tmps.tile([P, D], F32, tag="alpha_junk")
                        dot = stats.tile([P, 1], F32, tag="alpha_dot")
                        nc.vector.tensor_mul(out=junk, in0=xs[ct], in1=cen_b)
                        nc.vector.reduce_sum(out=dot, in_=junk, axis=mybir.AxisListType.X)
                        nc.scalar.activation(
                            out=alpha[:, ct : ct + 1],
                            in_=dot,
                            func=mybir.ActivationFunctionType.Sigmoid,
                        )

                for ch in range(2):  # c halves of 512
                    csl = slice(ch * (C // 2), (ch + 1) * (C // 2))
                    # --- ff1: h1[f, c] = relu(W1eff^T.T @ ht + b1) ---
                    for ft in range(FT):
                        if l == 0 and ch == 0 and ft < len(w1_pre):
                            w1c = w1_pre[ft]
                        else:
                            w1c = w1pool.tile([P, DT, P], BF16, tag="w1c")
                            nc.sync.dma_start(out=w1c, in_=w1_d[l, ft])
                        pt = ps1.tile([P, 512], F32, tag="ps1")
                        for dt in range(DT):
                            nc.tensor.matmul(
                                pt,
                                lhsT=w1c[:, dt, :],
                                rhs=ht[:, dt, csl],
                                start=(dt == 0),
                                stop=(dt == DT - 1),
                            )
                        nc.scalar.activation(
                            out=h1[:, ft, :],
                            in_=pt,
                            func=mybir.ActivationFunctionType.Relu,
                            bias=b1_sb[:, l, ft : ft + 1],
                            scale=1.0,
                        )
                    if ch == 0 and l > 0:
                        # issue the layer's W2 load while PE chews on ff1
                        nc.sync.dma_start(out=w2_sb, in_=w2_d[l])
                    # --- ff2 + residual (+ b2) ---
                    for ctl in range(CT // 2):
                        ct = ch * (CT // 2) + ctl
                        for dh in range(2):
                            dsl = slice(dh * 512, (dh + 1) * 512)
                            pt2 = ps2.tile([P, 512], F32, tag="ps2")
                            for ft in range(FT):
                                nc.tensor.matmul(
                                    pt2,
                                    lhsT=h1[:, ft, ctl * P : (ctl + 1) * P],
                                    rhs=w2_sb[:, ft, dsl],
                                    start=(ft == 0),
                                    stop=(ft == FT - 1),
                                )
                            nc.vector.tensor_add(
                                out=xs[ct][:, dsl], in0=xs[ct][:, dsl], in1=pt2
                            )
                            nc.vector.tensor_add(
                                out=xs[ct][:, dsl],
                                in0=xs[ct][:, dsl],
                                in1=b2_b[:, l, dsl],
                            )
                        if l == L - 1:
                            # final gate + output as soon as this ct is done
                            nc.vector.tensor_scalar_mul(
                                out=xs[ct],
                                in0=xs[ct],
                                scalar1=alpha[:, ct : ct + 1],
                            )
                            nc.gpsimd.dma_start(
                                out=y_d[ct * P : (ct + 1) * P, :], in_=xs[ct]
                            )
    if split_waits:
        _split_multi_waits(nc)
    return nc


_NC_CACHE = None


def _get_nc():
    global _NC_CACHE
    if _NC_CACHE is None:
        _NC_CACHE = build_bass()
    return _NC_CACHE


# ---------------------------------------------------------------------------
# Host side: routing (replicated, bit-exact with the reference) + sharding
# ---------------------------------------------------------------------------
def _routing_perm(features, centroids):
    # Replicates the reference's _balanced_assignment with the exact same jax
    # ops, pinned to the CPU backend: the reference itself can only run on
    # CPU jax (stable sort doesn't compile for the neuron backend), so CPU
    # numerics are the ones the permutation must match bit-for-bit.
    import jax
    import jax.numpy as jnp

    with jax.default_device(jax.devices("cpu")[0]):
        feats = jnp.asarray(features)
        cents = jnp.asarray(centroids)
        aff = jax.lax.stop_gradient(feats) @ jax.lax.stop_gradient(cents).T
        aff = jnp.nan_to_num(aff)
        capacity = feats.shape[0] // cents.shape[0]
        order = jnp.argsort(-aff.max(axis=1))
        aff_ord = aff[order]

        def step(counts, row):
            masked = jnp.where(counts < capacity, row, -jnp.inf)
            e = jnp.argmax(masked).astype(jnp.int32)
            return counts.at[e].add(1), e

        _, assign_ord = jax.lax.scan(
            step, jnp.zeros(cents.shape[0], jnp.int32), aff_ord
        )
        assign = jnp.zeros(feats.shape[0], jnp.int32).at[order].set(assign_ord)
        return np.asarray(jnp.argsort(assign))


def _prep_core_inputs(xr, centroids, ln_gamma, ln_beta, W1, b1, W2, b2):
    """Per-core input maps; folds gamma/beta into W1/b1 and pre-tiles weights."""
    maps = []
    for e in range(E):
        m = {"x": np.ascontiguousarray(xr[e])}
        w1s = np.empty((L, FT, P, DT, P), np.float32)
        w2s = np.empty((L, P, FT, D), np.float32)
        b1s = np.empty((L, P, FT), np.float32)
        for l in range(L):
            g = ln_gamma[l, e]
            bt = ln_beta[l, e]
            w1_eff = W1[l, e] * g[None, :]          # [F, D]
            b1_eff = b1[l, e] + W1[l, e] @ bt       # [F]
            # lhsT tiles: w1s[l, ft, p_d, dt, j_f] = w1_eff[ft*P+j, dt*P+p]
            w1s[l] = w1_eff.reshape(FT, P, DT, P).transpose(0, 3, 2, 1)
            # w2s[l, p_f, ft, d] = W2[l,e][d, ft*P+p]
            w2s[l] = W2[l, e].T.reshape(FT, P, D).transpose(1, 0, 2)
            b1s[l] = b1_eff.reshape(FT, P).T
        import ml_dtypes

        m["w1"] = w1s.astype(ml_dtypes.bfloat16)
        m["w2"] = w2s.astype(ml_dtypes.bfloat16)
        m["b1"] = b1s
        m["b2"] = np.ascontiguousarray(b2[:, e, :]).astype(np.float32)
        m["cen"] = np.ascontiguousarray(centroids[e]).astype(np.float32)
        maps.append(m)
    return maps


def kernel(
    input_features,
    centroids,
    ln_gamma,
    ln_beta,
    W1,
    b1,
    W2,
    b2,
    input_ids=None,
    _trace=False,
    _tmpdir=None,
):
    input_features = np.asarray(input_features, np.float32)
    centroids = np.asarray(centroids, np.float32)
    ln_gamma = np.asarray(ln_gamma, np.float32)
    ln_beta = np.asarray(ln_beta, np.float32)
    W1 = np.asarray(W1, np.float32)
    b1 = np.asarray(b1, np.float32)
    W2 = np.asarray(W2, np.float32)
    b2 = np.asarray(b2, np.float32)

    feats = input_features.reshape(T, D)
    perm = _routing_perm(feats, centroids)
    xr = feats[perm].reshape(E, C, D)

    maps = _prep_core_inputs(xr, centroids, ln_gamma, ln_beta, W1, b1, W2, b2)
    nc = _get_nc()
    res = run_bass_kernel_spmd(
        nc, maps, list(range(E)), trace=_trace, tmpdir=_tmpdir
    )
    y = np.concatenate([res.results[e]["y"] for e in range(E)], axis=0)  # [T, D]
    out = np.zeros((T, D), np.float32)
    out[perm] = y
    out = out.reshape(input_features.shape)
    if _trace:
        return out, res
    return out
